# revision 1
# baseline (speedup 1.0000x reference)
"""FlowerAttention Trainium2 kernel (8 NeuronCores, tensor-parallel).

Problem: y = proj(attn(rmsnorm+rope(qkv(x)))) with
  x [4, 2048, 2048], w_qkv [6144, 2048], w_proj [2048, 2048],
  per-head RMSNorm on q/k (head_dim=128, eps 1e-6), half-split RoPE
  (theta=32), dense softmax attention (no mask), output projection.

Sharding: core c -> (batch b = c//2, head-group g = c%2 of 8 heads).
Each core computes the full pipeline for its (b, g); the output
projection contracts only the local 1024 head-dims, so the host sums
the two partial outputs per batch.

Device-side layout strategy (all matmuls in float32r = full-rate fp32):
 - Host pre-transposes x -> xT [d, s] and weight slices -> wT [d, e].
 - Phase V/Q/K: QKV matmuls produce V in natural [s, e] layout and
   Q^T/K^T in [head_dim, s] layout directly (no transposes on device).
   RMSNorm is folded as a per-(position,head) scalar applied after RoPE
   (RoPE is a rotation, so it commutes with the scalar); the norm
   weights are folded into host-precomputed cos/sin tables. The
   sum-of-squares over head_dim (the partition axis) is done with a
   ones-matmul on the PE, which also replicates it across partitions.
   The RoPE half-swap is a single permutation-matmul on the PE.
 - Phase attention (per head): E^T tile [k_seq, q_seq-block] =
   exp(scale * K^T.T @ Q^T) via PE + ScalarE; row-sums r accumulate on
   VectorE + a ones-matmul; O^T [head_dim, q_seq] accumulates via
   lhsT=V_kt, rhs=E^T_kt, then is normalized by 1/r.
 - Phase proj: out[s, e] partial = sum_h O^T_h.T @ w_projT, straight
   from the O^T layout, with w_projT streamed in column blocks.

Weight/table loads for each phase are issued from pools opened before
the previous phase's pools so the DMAs overlap prior compute instead of
stalling the PE at phase boundaries.
"""

import os
import sys
from contextlib import ExitStack

for _p in (
    "/root/.axon_site",
    "/root/.axon_site/_ro/trn_rl_repo",
    "/root/.axon_site/_ro/pypackages",
    "/opt/trn_rl_repo",
):
    if os.path.isdir(_p) and _p not in sys.path:
        sys.path.append(_p)

import numpy as np

import concourse.bass as bass  # noqa: F401
import concourse.tile as tile
from concourse import bacc, mybir
from concourse.bass_utils import run_bass_kernel_spmd

F32 = mybir.dt.float32
F32R = mybir.dt.float32r
BF16 = mybir.dt.bfloat16
AF = mybir.ActivationFunctionType

DIM = 2048
N_HEADS = 16
HEAD_DIM = 128
SEQ = 2048
BSZ = 4
THETA = 32.0
EPS = 1e-6
N_CORES = 8
HPC = 8  # heads per core
LOCAL_E = HPC * HEAD_DIM  # 1024
SCALE = HEAD_DIM ** -0.5
P = 128
KT = DIM // P  # 16 contraction subtiles over model dim
SB = 512  # free-dim block
NSB = SEQ // SB  # 4 seq blocks
NST = SEQ // P  # 16 seq tiles

# E (attention weights) and V dtype.  bf16 halves VectorE row-sum work,
# SBUF footprint and scratch traffic; flip to F32R if accuracy demands.
E_DT = BF16


def _build_program(reps=1, extra=()):
    nc = bacc.Bacc()

    xt = nc.dram_tensor("xt", [DIM, SEQ], F32R, kind="ExternalInput")
    wq = nc.dram_tensor("wq", [DIM, LOCAL_E], F32R, kind="ExternalInput")
    wk = nc.dram_tensor("wk", [DIM, LOCAL_E], F32R, kind="ExternalInput")
    wv = nc.dram_tensor("wv", [DIM, LOCAL_E], F32R, kind="ExternalInput")
    wp = nc.dram_tensor("wp", [LOCAL_E, DIM], F32R, kind="ExternalInput")
    cq = nc.dram_tensor("cq", [P, SEQ], F32, kind="ExternalInput")
    sq = nc.dram_tensor("sq", [P, SEQ], F32, kind="ExternalInput")
    ck = nc.dram_tensor("ck", [P, SEQ], F32, kind="ExternalInput")
    sk = nc.dram_tensor("sk", [P, SEQ], F32, kind="ExternalInput")
    swap = nc.dram_tensor("swap", [P, P], F32R, kind="ExternalInput")
    out = nc.dram_tensor("out", [SEQ, DIM], F32, kind="ExternalOutput")

    xt_re = xt[:].rearrange("(kt p) s -> p kt s", p=P)
    wv_re = wv[:].rearrange("(kt p) e -> p kt e", p=P)
    wp_re = wp[:].rearrange("(h p) e -> p h e", p=P)

    with tile.TileContext(nc) as tc:
        outer_es = ExitStack()
        with outer_es:
            dram = outer_es.enter_context(
                tc.tile_pool(name="dram", bufs=1, space="DRAM")
            )
            const = outer_es.enter_context(tc.tile_pool(name="const", bufs=1))

            qT_s = dram.tile([HPC, P, SEQ], F32R)
            kT_s = dram.tile([HPC, P, SEQ], F32R)
            v_s = dram.tile([SEQ, LOCAL_E], E_DT)
            v_s_re = v_s[:].rearrange("(kt p) e -> p kt e", p=P)

            eps_t = const.tile([P, 1], F32)
            nc.vector.memset(eps_t[:], EPS)
            ones_f = const.tile([P, P], F32)
            nc.vector.memset(ones_f[:], 1.0)
            ones_r = const.tile([P, P], F32R)
            nc.vector.tensor_copy(ones_r[:], ones_f[:])
            swap_t = const.tile([P, P], F32R)
            nc.sync.dma_start(out=swap_t[:], in_=swap[:])

            for _rep in range(reps):
                _emit_body(
                    nc, tc, xt_re, wv_re, wp_re,
                    wq, wk, cq, sq, ck, sk, out,
                    qT_s, kT_s, v_s, v_s_re,
                    eps_t, ones_f, ones_r, swap_t,
                )
            for part in extra:
                _emit_body(
                    nc, tc, xt_re, wv_re, wp_re,
                    wq, wk, cq, sq, ck, sk, out,
                    qT_s, kT_s, v_s, v_s_re,
                    eps_t, ones_f, ones_r, swap_t,
                    parts=(part,),
                )

    nc.finalize()
    return nc


def _emit_body(
    nc, tc, xt_re, wv_re, wp_re,
    wq, wk, cq, sq, ck, sk, out,
    qT_s, kT_s, v_s, v_s_re,
    eps_t, ones_f, ones_r, swap_t,
    parts=("qkv", "attn"),
):
        es = ExitStack()
        with es:
            # One shared weight/table pool for the Q and K passes: the K
            # tiles use the same tags, so their loads begin as soon as the
            # Q pass releases each slot (overlapping the Q tail) instead of
            # stalling at the phase boundary.
            # Prefetch slot (xt first-half, later head-0 q^T).  Opened
            # before the w/cs pools so the LIFO pool-stack order holds when
            # qk_es closes while this pool lives into the attention phase.
            xt0_pool = es.enter_context(tc.tile_pool(name="xt0", bufs=1))

            qk_es = ExitStack()
            w_pool = qk_es.enter_context(tc.tile_pool(name="w_qk", bufs=1))
            cs_pool = qk_es.enter_context(tc.tile_pool(name="cs_qk", bufs=2))

            def load_w_cs(w_dram, c_dram, s_dram):
                # two 4-head halves -> 2KB DMA lines instead of 512B
                w_re = w_dram[:].rearrange("(kt p) e -> p kt e", p=P)
                w_halves = []
                for i in range(2):
                    wt = w_pool.tile([P, KT, LOCAL_E // 2], F32R, tag=f"w{i}")
                    nc.sync.dma_start(
                        out=wt[:],
                        in_=w_re[:, :, i * (LOCAL_E // 2) : (i + 1) * (LOCAL_E // 2)],
                    )
                    w_halves.append(wt)
                w_sb = [
                    w_halves[h // 4][:, :, (h % 4) * P : (h % 4 + 1) * P]
                    for h in range(HPC)
                ]
                ctab = cs_pool.tile([P, SEQ], F32, tag="ctab")
                nc.sync.dma_start(out=ctab[:], in_=c_dram[:])
                stab = cs_pool.tile([P, SEQ], F32, tag="stab")
                nc.sync.dma_start(out=stab[:], in_=s_dram[:])
                return w_sb, ctab, stab

            # Prefetch Q-pass weights/tables during the V phase.  The
            # negative-offset priority sorts these DMAs after the V-phase
            # work so they don't steal bandwidth from the critical first
            # wv/xt loads.
            with tc.high_priority(offset=-50000):
                q_w, q_ctab, q_stab = load_w_cs(wq, cq, sq)
                # First half of the Q pass's first xt block, prefetched so
                # the Q pass has PE work while the rest of xt streams in.
                # The slot is later reused to preload head 0's q^T.
                xt0a = xt0_pool.tile([P, KT // 2, SB], F32R, tag="pre")
                nc.sync.dma_start(out=xt0a[:], in_=xt_re[:, : KT // 2, :SB])

            # ---------------- Phase V: v = x @ wv^T (natural layout) -------
            with (
                tc.tile_pool(name="wv_pool", bufs=1) as wv_pool,
                tc.tile_pool(name="v_tmp", bufs=3) as v_tmp,
                tc.tile_pool(name="v_psum", bufs=4, space="PSUM") as v_psum,
            ):
                wv_sb = wv_pool.tile([P, KT, LOCAL_E], F32R)
                nc.sync.dma_start(out=wv_sb[:], in_=wv_re)
                for st in range(NST):
                    xt_col = v_tmp.tile([P, KT, P], F32R, tag="xtc")
                    nc.sync.dma_start(
                        out=xt_col[:], in_=xt_re[:, :, st * P : (st + 1) * P]
                    )
                    for vb in range(LOCAL_E // SB):
                        ps_v = v_psum.tile([P, SB], F32)
                        for kt in range(KT):
                            nc.tensor.matmul(
                                ps_v[:],
                                xt_col[:, kt, :],
                                wv_sb[:, kt, vb * SB : (vb + 1) * SB],
                                start=(kt == 0),
                                stop=(kt == KT - 1),
                            )
                        vsb = v_tmp.tile([P, SB], E_DT, tag="vsb")
                        nc.scalar.activation(
                            out=vsb[:], in_=ps_v[:], func=AF.Copy
                        )
                        nc.sync.dma_start(
                            out=v_s[st * P : (st + 1) * P, vb * SB : (vb + 1) * SB],
                            in_=vsb[:],
                        )


            # ---------------- Phases Q / K: transposed + RMS + RoPE --------
            with (
                tc.tile_pool(name="x_pool", bufs=2) as x_pool,
                tc.tile_pool(name="t_pool", bufs=2) as t_pool,
                tc.tile_pool(name="ps_a", bufs=3, space="PSUM") as ps_a,
                tc.tile_pool(name="ps_b", bufs=2, space="PSUM") as ps_b,
                tc.tile_pool(name="ps_c", bufs=2, space="PSUM") as ps_c,
            ):
                HKT = KT // 2

                def qk_pass(w_sb, ctab, stab, dst, xt_first=None):
                    for sb in range(NSB):
                        ss = slice(sb * SB, (sb + 1) * SB)
                        if sb == 0 and xt_first is not None:
                            xt_lo = xt_first
                        else:
                            xt_lo = x_pool.tile([P, HKT, SB], F32R, tag="xlo")
                            nc.sync.dma_start(
                                out=xt_lo[:], in_=xt_re[:, :HKT, ss]
                            )
                        xt_hi = x_pool.tile([P, HKT, SB], F32R, tag="xhi")
                        nc.sync.dma_start(out=xt_hi[:], in_=xt_re[:, HKT:, ss])
                        for h in range(HPC):
                            ps_q = ps_a.tile([P, SB], F32)
                            for kt in range(KT):
                                xt_kt = (
                                    xt_lo[:, kt, :]
                                    if kt < HKT
                                    else xt_hi[:, kt - HKT, :]
                                )
                                nc.tensor.matmul(
                                    ps_q[:],
                                    w_sb[h][:, kt, :],
                                    xt_kt,
                                    start=(kt == 0),
                                    stop=(kt == KT - 1),
                                )
                            qt = t_pool.tile([P, SB], F32R, tag="qt")
                            nc.scalar.activation(
                                out=qt[:], in_=ps_q[:], func=AF.Copy
                            )
                            sqt = t_pool.tile([P, SB], F32R, tag="sqt")
                            nc.scalar.activation(
                                out=sqt[:], in_=ps_q[:], func=AF.Square
                            )
                            ps_ms = ps_b.tile([P, SB], F32)
                            nc.tensor.matmul(
                                ps_ms[:], ones_r[:], sqt[:], start=True, stop=True
                            )
                            rms = t_pool.tile([P, SB], F32, tag="rms")
                            nc.scalar.activation(
                                out=rms[:],
                                in_=ps_ms[:],
                                func=AF.Sqrt,
                                scale=1.0 / HEAD_DIM,
                                bias=eps_t[:],
                            )
                            inv = t_pool.tile([P, SB], F32, tag="inv")
                            nc.vector.reciprocal(inv[:], rms[:])
                            ps_rot = ps_c.tile([P, SB], F32)
                            nc.tensor.matmul(
                                ps_rot[:], swap_t[:], qt[:], start=True, stop=True
                            )
                            t1 = t_pool.tile([P, SB], F32, tag="t1")
                            nc.vector.tensor_mul(
                                t1[:], qt[:].bitcast(F32), ctab[:, ss]
                            )
                            t2 = t_pool.tile([P, SB], F32, tag="t2")
                            nc.vector.tensor_mul(t2[:], ps_rot[:], stab[:, ss])
                            nc.vector.tensor_add(t1[:], t1[:], t2[:])
                            qr = t_pool.tile([P, SB], F32R, tag="qr")
                            nc.vector.tensor_mul(qr[:], t1[:], inv[:])
                            nc.sync.dma_start(out=dst[h, :, ss], in_=qr[:])

                qk_pass(q_w, q_ctab, q_stab, qT_s, xt_first=xt0a)
                # The xt0a slot frees after Q's first block; reuse it to
                # preload head 0's q^T so attention starts without a stall.
                qh0 = xt0_pool.tile([P, SEQ], F32R, tag="pre")
                nc.sync.dma_start(out=qh0[:], in_=qT_s[0])
                # K tiles reuse the Q slots; loads overlap the Q tail.
                k_w, k_ctab, k_stab = load_w_cs(wk, ck, sk)
                qk_pass(k_w, k_ctab, k_stab, kT_s)
            qk_es.close()

            # ---------------- Phase attention + proj -----------------------
            with (
                tc.tile_pool(name="oT_pool", bufs=1) as oT_pool,
                tc.tile_pool(name="head_pool", bufs=2) as head_pool,
                tc.tile_pool(name="e_pool", bufs=2) as e_pool,
                tc.tile_pool(name="a_tmp", bufs=2) as a_tmp,
                tc.tile_pool(name="ap_tmp", bufs=1) as ap_tmp,
                tc.tile_pool(name="wp_pool", bufs=2) as wp_pool,
                tc.tile_pool(name="p_tmp", bufs=3) as p_tmp,
                tc.tile_pool(name="ps_e", bufs=2, space="PSUM") as ps_e_pool,
                tc.tile_pool(name="ps_r", bufs=2, space="PSUM") as ps_r_pool,
                tc.tile_pool(name="ps_o", bufs=2, space="PSUM") as ps_o_pool,
            ):
                oT = oT_pool.tile([P, HPC, SEQ], F32R)
                for h in range(HPC):
                    if h == 0:
                        qh = qh0
                    else:
                        qh = head_pool.tile([P, SEQ], F32R, tag="qh")
                        nc.sync.dma_start(out=qh[:], in_=qT_s[h])
                    kh = head_pool.tile([P, SEQ], F32R, tag="kh")
                    nc.sync.dma_start(out=kh[:], in_=kT_s[h])
                    vh = head_pool.tile([P, KT, P], E_DT, tag="vh")
                    nc.sync.dma_start(
                        out=vh[:], in_=v_s_re[:, :, h * P : (h + 1) * P]
                    )
                    for qb in range(NSB):
                        qs = slice(qb * SB, (qb + 1) * SB)
                        e_all = e_pool.tile([P, KT, SB], E_DT, tag="eall")
                        racc2 = a_tmp.tile([P, 2, SB], F32R, tag="racc2")
                        racc2p = ap_tmp.tile([P, 2, SB], F32R, tag="racc2p")
                        for kt2 in range(KT // 2):
                            # pair of k-tiles -> one 2-bank psum tile so the
                            # exp (the attention-phase bottleneck) runs as a
                            # single [128, 1024] ScalarE op
                            ps_e = ps_e_pool.tile([P, 2, SB], F32)
                            for j in range(2):
                                kt = 2 * kt2 + j
                                nc.tensor.matmul(
                                    ps_e[:, j, :],
                                    kh[:, kt * P : (kt + 1) * P],
                                    qh[:, qs],
                                    start=True,
                                    stop=True,
                                )
                            nc.scalar.activation(
                                out=e_all[:, 2 * kt2 : 2 * kt2 + 2, :],
                                in_=ps_e[:],
                                func=AF.Exp,
                                scale=SCALE,
                            )
                            if kt2 == 0:
                                nc.vector.tensor_copy(
                                    racc2[:], e_all[:, 0:2, :]
                                )
                            else:
                                nc.vector.tensor_add(
                                    racc2[:],
                                    racc2[:],
                                    e_all[:, 2 * kt2 : 2 * kt2 + 2, :],
                                )
                        nc.vector.tensor_add(
                            racc2[:, 0, :], racc2[:, 0, :], racc2[:, 1, :]
                        )
                        ps_rr = ps_r_pool.tile([P, SB], F32)
                        nc.tensor.matmul(
                            ps_rr[:],
                            ones_r[:],
                            racc2[:, 0, :],
                            start=True,
                            stop=True,
                        )
                        invr = a_tmp.tile([P, SB], F32, tag="invr")
                        nc.vector.reciprocal(invr[:], ps_rr[:])
                        ps_o = ps_o_pool.tile([P, SB], F32)
                        for kt in range(KT):
                            nc.tensor.matmul(
                                ps_o[:],
                                vh[:, kt, :],
                                e_all[:, kt, :],
                                start=(kt == 0),
                                stop=(kt == KT - 1),
                            )
                        nc.vector.tensor_mul(oT[:, h, qs], ps_o[:], invr[:])

                # proj: stream w_projT column blocks; psum shared with ps_r
                for eb in range(NSB):
                    es_ = slice(eb * SB, (eb + 1) * SB)
                    wp_eb = wp_pool.tile([P, HPC, SB], F32R, tag="wpeb")
                    nc.sync.dma_start(out=wp_eb[:], in_=wp_re[:, :, es_])
                    for st in range(NST):
                        ps_p = ps_r_pool.tile([P, SB], F32, tag="ps_rr")
                        for h in range(HPC):
                            nc.tensor.matmul(
                                ps_p[:],
                                oT[:, h, st * P : (st + 1) * P],
                                wp_eb[:, h, :],
                                start=(h == 0),
                                stop=(h == HPC - 1),
                            )
                        ob = p_tmp.tile([P, SB], F32, tag="ob")
                        nc.scalar.activation(
                            out=ob[:], in_=ps_p[:], func=AF.Copy
                        )
                        nc.sync.dma_start(
                            out=out[st * P : (st + 1) * P, es_], in_=ob[:]
                        )


_PROGRAM = None


def _get_program():
    global _PROGRAM
    if _PROGRAM is None:
        _PROGRAM = _build_program()
    return _PROGRAM


def _rope_tables(norm_w):
    """C/S tables [128, SEQ] for transposed-layout RoPE with the per-head
    norm weight folded in.  out = q*C + rot(q)*S with rot(q)[p] =
    q[(p+64) % 128]."""
    half = HEAD_DIM // 2  # 64
    freqs = THETA ** (-np.arange(0, HEAD_DIM, 2, dtype=np.float32) / HEAD_DIM)
    pos = np.arange(SEQ, dtype=np.float32)
    ang = pos[:, None] * freqs[None, :]  # [SEQ, 64]
    cos = np.cos(ang).astype(np.float32)  # [SEQ, 64]
    sin = np.sin(ang).astype(np.float32)
    w = np.asarray(norm_w, dtype=np.float32)
    C = np.empty((P, SEQ), dtype=np.float32)
    S = np.empty((P, SEQ), dtype=np.float32)
    for p in range(P):
        C[p] = cos[:, p % half] * w[p]
    for p in range(half):
        S[p] = -sin[:, p] * w[p + half]
    for p in range(half, P):
        S[p] = sin[:, p - half] * w[p - half]
    return C, S


def kernel(x, w_qkv, w_proj, q_norm_w, k_norm_w):
    x = np.asarray(x, dtype=np.float32)
    w_qkv = np.asarray(w_qkv, dtype=np.float32)
    w_proj = np.asarray(w_proj, dtype=np.float32)

    nc = _get_program()

    cq_t, sq_t = _rope_tables(q_norm_w)
    ck_t, sk_t = _rope_tables(k_norm_w)
    swap_m = np.zeros((P, P), dtype=np.float32)
    half = HEAD_DIM // 2
    for p in range(P):
        swap_m[(p + half) % P, p] = 1.0

    xts = [np.ascontiguousarray(x[b].T) for b in range(BSZ)]
    wqs, wks, wvs, wps = [], [], [], []
    for g in range(2):
        rows = slice(g * LOCAL_E, (g + 1) * LOCAL_E)
        wqs.append(np.ascontiguousarray(w_qkv[rows].T))
        wks.append(np.ascontiguousarray(w_qkv[DIM + g * LOCAL_E : DIM + (g + 1) * LOCAL_E].T))
        wvs.append(np.ascontiguousarray(w_qkv[2 * DIM + g * LOCAL_E : 2 * DIM + (g + 1) * LOCAL_E].T))
        wps.append(np.ascontiguousarray(w_proj[:, rows].T))

    in_maps = []
    for c in range(N_CORES):
        b, g = c // 2, c % 2
        in_maps.append(
            {
                "xt": xts[b],
                "wq": wqs[g],
                "wk": wks[g],
                "wv": wvs[g],
                "wp": wps[g],
                "cq": cq_t,
                "sq": sq_t,
                "ck": ck_t,
                "sk": sk_t,
                "swap": swap_m,
            }
        )

    res = run_bass_kernel_spmd(nc, in_maps, list(range(N_CORES)))
    out = np.empty((BSZ, SEQ, DIM), dtype=np.float32)
    for b in range(BSZ):
        out[b] = res.results[2 * b]["out"] + res.results[2 * b + 1]["out"]
    return out

